# revision 3
# baseline (speedup 1.0000x reference)
"""Trainium2 Bass kernel for a pre-norm transformer block (MHA + MLP).

Sharding: sequence-parallel over 8 cores (batch b = core//4, token block
core%4, 512 tokens each). Weights replicated. One 4-rank AllGather per
batch group moves K+V (fp8, 1MB in / 4MB out).

Dataflow is feature-major end-to-end (channels on partitions, tokens on
the free axis), no on-chip transposes. Precision plan:
  - attention path in fp8e4m3 with DoubleRow matmuls (0.5 cycles/row):
    LN1 stats, QKV, scores (zero-padded pairs), P*V (true kc pairs), proj.
    Softmax averaging keeps the fp8 noise out of the residual stream.
  - MLP in fp16 (1 cycle/row at any tile size, halves weight DMA).
  - residual stream x2 kept in fp16/fp32; LN affines folded into the
    adjacent weights (exact); softmax exp uses scale=1/8, bias=-3 to stay
    inside e4m3 range; rstd = exp(-0.5*ln(var+eps)) so the Activation
    engine never leaves the exp/ln table during the softmax stream.
  - two query-half pipeline: attention(qh1) exps overlap MLP(qh0) matmuls.
"""
import sys

sys.path.insert(0, "/opt/trn_rl_repo")
import numpy as np
import ml_dtypes
import concourse.bass as bass
import concourse.mybir as mybir
import concourse.tile as tile
from concourse import bacc
from concourse.bass_utils import run_bass_kernel_spmd

# problem shapes (hardcoded per contract)
B, N, D = 2, 2048, 1024
H, DH = 16, 64
HID = 4096
NCORES = 8
TOK = (B * N) // NCORES  # 512 tokens per core
EPS = 1e-5
SCALE = DH**-0.5
P = 128
CH = D // P        # 8 channel chunks of the model dim
KC = N // P        # 16 key chunks of the full sequence
HCH = HID // P     # 32 hidden chunks
RANKS = 4          # per-batch replica group size
NQH = 2            # query halves (pipeline granularity)
QT = TOK // NQH    # 256 tokens per half
EXPB = -4.0        # softmax exp bias (softmax shift-invariant)

F32 = mybir.dt.float32
F16 = mybir.dt.float16
F8 = mybir.dt.float8e4
F8E5 = mybir.dt.float8e5
AF = mybir.ActivationFunctionType
OP = mybir.AluOpType
DR = mybir.MatmulPerfMode.DoubleRow

NP_F8 = ml_dtypes.float8_e4m3

REPLICA_GROUPS = [[0, 1, 2, 3], [4, 5, 6, 7]]


def build_program(do_compile=True):
    nc = bacc.Bacc("TRN2", target_bir_lowering=False, debug=False, num_devices=NCORES)

    # ---- kernel I/O ----
    xT = nc.dram_tensor("xT", [D, TOK], F32, kind="ExternalInput").ap()
    xT16_d = nc.dram_tensor("xT16", [P, CH, TOK], F16, kind="ExternalInput").ap()
    wqkv_d = nc.dram_tensor("wqkv", [P, CH, 3 * D], F8, kind="ExternalInput").ap()
    wproj_d = nc.dram_tensor("wproj", [P, CH, D], F8, kind="ExternalInput").ap()
    wfc1_d = nc.dram_tensor("wfc1", [P, CH, HID], F16, kind="ExternalInput").ap()
    wfc2_d = nc.dram_tensor("wfc2", [CH, P, HCH, P], F16, kind="ExternalInput").ap()
    qb_d = nc.dram_tensor("qb", [P, CH], F32, kind="ExternalInput").ap()
    kb_d = nc.dram_tensor("kb", [P, CH], F32, kind="ExternalInput").ap()
    projb_d = nc.dram_tensor("projb", [P, CH], F32, kind="ExternalInput").ap()
    fc1b_d = nc.dram_tensor("fc1b", [P, HCH], F32, kind="ExternalInput").ap()
    fc2b_d = nc.dram_tensor("fc2b", [P, CH], F32, kind="ExternalInput").ap()
    outT = nc.dram_tensor("outT", [D, TOK], F32, kind="ExternalOutput").ap()

    xT_chunks = xT.rearrange("(ch p) t -> p ch t", p=P)

    with tile.TileContext(nc) as tc:
        with (
            tc.tile_pool(name="consts", bufs=1) as consts,
            tc.tile_pool(name="wbig", bufs=1) as wbig,
            tc.tile_pool(name="w2p", bufs=2) as w2p,
            tc.tile_pool(name="b4k", bufs=3) as b4k,
            tc.tile_pool(name="res", bufs=1) as res,
            tc.tile_pool(name="kv", bufs=2) as kvp,
            tc.tile_pool(name="vfp", bufs=1) as vfp,
            tc.tile_pool(name="cxp", bufs=2) as cxp,
            tc.tile_pool(name="xc3", bufs=2) as xc3,
            tc.tile_pool(name="stg", bufs=2) as stg,
            tc.tile_pool(name="rows", bufs=2) as rows,
            tc.tile_pool(name="bc", bufs=2) as bcp,
            tc.tile_pool(name="dram", bufs=1, space="DRAM") as dram,
        ):
            # ---- constants ----
            ones8 = consts.tile([P, 1], F8, tag="ones8")
            nc.vector.memset(ones8[:], 1.0)
            eps_row = consts.tile([1, 1], F32, tag="eps")
            nc.vector.memset(eps_row[:], EPS)
            expb_row = consts.tile([P, 1], F32, tag="expb")
            nc.vector.memset(expb_row[:], EXPB)
            ones16 = consts.tile([P, 1], F16, tag="ones16")
            nc.vector.memset(ones16[:], 1.0)
            ones8d = consts.tile([P, 2, 16], F8, tag="ones8d")
            nc.vector.memset(ones8d[:], 1.0)
            qb_sb = consts.tile([P, CH], F32, tag="qb")
            kb_sb = consts.tile([P, CH], F32, tag="kb")
            projb_sb = consts.tile([P, CH], F32, tag="projb")
            fc1b_sb = consts.tile([P, HCH], F32, tag="fc1b")
            fc2b_sb = consts.tile([P, CH], F32, tag="fc2b")
            for t_, s_ in (
                (qb_sb, qb_d), (kb_sb, kb_d), (projb_sb, projb_d),
                (fc1b_sb, fc1b_d), (fc2b_sb, fc2b_d),
            ):
                nc.gpsimd.dma_start(t_[:], s_[:])

            # ---- resident weights (wproj/wfc1 DMAs issued later: they are
            # needed only ~100us in, and the DMA device is serialized) ----
            wqkv = wbig.tile([P, CH, 3 * D], F8, tag="wqkv")
            wproj = wbig.tile([P, CH, D], F8, tag="wproj")
            wfc1 = wbig.tile([P, CH, HID], F16, tag="wfc1")

            # ---- collective DRAM buffers (K rows 0:1024 c-major, V rows
            # 1024:2048 token-major in vout halves) ----
            kv_in = dram.tile([2 * D, TOK], F8, tag="kvin")
            kv_out = dram.tile([RANKS * 2 * D, TOK], F8, tag="kvout")
            kvi = kv_in[:].rearrange("(a p) t -> a p t", p=P)  # a = 16 blocks
            kvo = kv_out[:].rearrange("(r a p) t -> r a p t", r=RANKS, p=P)

            # ================= front: LN1 =================
            # x arrives once as a host-prepared fp16 copy in the exact SBUF
            # layout (one fat DMA instead of 16 chunk loads + copies)
            xf16 = res.tile([P, CH, TOK], F16, tag="xf16")
            nc.sync.dma_start(xf16[:], xT16_d[:])

            with tc.tile_pool(name="prow", bufs=2, space="PSUM") as prow:
                ps_mu = prow.tile([1, TOK], F32, tag="row", name="ps_mu1")
                ps_s2 = prow.tile([1, TOK], F32, tag="row", name="ps_s21")
                for ch in range(CH):
                    sq = stg.tile([P, TOK], F16, tag="t1", name=f"sq16_{ch}")
                    nc.vector.tensor_mul(sq[:], xf16[:, ch, :], xf16[:, ch, :])
                    nc.tensor.matmul(
                        ps_mu[:], ones16[:], xf16[:, ch, :],
                        start=(ch == 0), stop=(ch == CH - 1),
                    )
                    nc.tensor.matmul(
                        ps_s2[:], ones16[:], sq[:],
                        start=(ch == 0), stop=(ch == CH - 1),
                    )
                mu = rows.tile([1, TOK], F16, tag="r", bufs=5, name="mu1")
                nc.vector.tensor_scalar_mul(mu[:], ps_mu[:], 1.0 / D)
                var = rows.tile([1, TOK], F16, tag="r", bufs=5, name="var1")
                nc.vector.tensor_tensor(var[:], mu[:], mu[:], OP.mult)
                ex2 = rows.tile([1, TOK], F16, tag="r", bufs=5, name="ex21")
                nc.vector.tensor_scalar_mul(ex2[:], ps_s2[:], 1.0 / D)
                nc.vector.tensor_sub(var[:], ex2[:], var[:])
            # rstd = exp(-0.5*ln(var+eps)) — stays in the exp/ln ACT table
            nc.scalar.activation(out=ex2[:], in_=var[:], func=AF.Ln, bias=eps_row[:])
            rstd = var
            nc.scalar.activation(out=rstd[:], in_=ex2[:], func=AF.Exp, scale=-0.5)
            nc.vector.tensor_tensor(mu[:], mu[:], rstd[:], OP.mult)  # cpos in place
            rstd_b = bcp.tile([P, TOK], F16, tag="bc", name="rstdb1")
            nc.gpsimd.partition_broadcast(rstd_b[:], rstd[:])
            c_b = bcp.tile([P, TOK], F16, tag="bc", name="cb1")
            nc.gpsimd.partition_broadcast(c_b[:], mu[:])

            h1 = b4k.tile([P, CH, TOK], F8, tag="b4k", name="h1")
            for ch in range(CH):
                t1 = stg.tile([P, TOK], F16, tag="t1", name=f"h1t{ch}")
                eng = nc.vector if ch % 2 == 0 else nc.gpsimd
                eng.tensor_mul(t1[:], xf16[:, ch, :], rstd_b[:])
                eng.tensor_sub(h1[:, ch, :], t1[:], c_b[:])

            # ================= front: QKV (fp8 DoubleRow) =================
            # weight DMAs chunked to ~1-2us device holds so they cannot
            # head-of-line-block latency-critical loads
            for wc in range(4):
                nc.sync.dma_start(
                    wqkv[:, 2 * wc : 2 * wc + 2, :], wqkv_d[:, 2 * wc : 2 * wc + 2, :]
                )
            fpool_cm = tc.tile_pool(name="ps_f", bufs=8, space="PSUM")
            fps = fpool_cm.__enter__()

            def qkv_psum(col0, name):
                ps = fps.tile([P, TOK], F32, tag="acc", name=name)
                for ch in range(0, CH, 2):
                    nc.tensor.matmul(
                        ps[:], wqkv[:, ch : ch + 2, col0 : col0 + P],
                        h1[:, ch : ch + 2, :],
                        start=(ch == 0), stop=(ch == CH - 2), perf_mode=DR,
                    )
                return ps

            # V first (token-major), drains alternating DVE/ACT so neither
            # engine paces the collective's input
            for vh in range(2):
                for tt in range(TOK // P):
                    ps = fps.tile([P, TOK], F32, tag="acc", name=f"v{vh}_{tt}")
                    for ch in range(0, CH, 2):
                        nc.tensor.matmul(
                            ps[:],
                            h1[:, ch : ch + 2, tt * P : (tt + 1) * P],
                            wqkv[:, ch : ch + 2, 2 * D + vh * TOK : 2 * D + (vh + 1) * TOK],
                            start=(ch == 0), stop=(ch == CH - 2), perf_mode=DR,
                        )
                    vtmp = stg.tile([P, TOK], F8, tag="cp", bufs=6, name=f"vtmp{vh}_{tt}")
                    if tt % 2 == 0:
                        nc.scalar.activation(out=vtmp[:], in_=ps[:], func=AF.Identity)
                    else:
                        nc.vector.tensor_copy(out=vtmp[:], in_=ps[:])
                    nc.sync.dma_start(kvi[CH + vh * (TOK // P) + tt, :, :], vtmp[:])
            for m in range(CH):
                ps = qkv_psum(D + m * P, f"k{m}")
                ktmp = stg.tile([P, TOK], F8, tag="cp", bufs=6, name=f"ktmp{m}")
                if m % 2 == 0:
                    nc.scalar.activation(
                        out=ktmp[:], in_=ps[:], func=AF.Identity,
                        bias=kb_sb[:, m : m + 1],
                    )
                else:
                    nc.vector.tensor_scalar(
                        out=ktmp[:], in0=ps[:], scalar1=kb_sb[:, m : m + 1],
                        scalar2=None, op0=OP.add,
                    )
                nc.sync.dma_start(kvi[m, :, :], ktmp[:])
            nc.gpsimd.collective_compute(
                "AllGather", OP.bypass,
                ins=[kv_in[:].opt()], outs=[kv_out[:].opt()],
                replica_groups=REPLICA_GROUPS,
            )
            # Q into qzT (slots 0..7 = q per m-group, slot 8 = shared zeros
            # that kill the second DoubleRow member of the score matmuls)
            qzT = res.tile([P, CH + 1, TOK], F8, tag="qzT")
            nc.gpsimd.memset(qzT[:, CH, :], 0.0)
            for m in range(CH):
                ps = qkv_psum(m * P, f"q{m}")
                nc.scalar.activation(
                    out=qzT[:, m, :], in_=ps[:], func=AF.Identity,
                    bias=qb_sb[:, m : m + 1],
                )
            fpool_cm.__exit__(None, None, None)

            # K tiles resident for all query chunks (loaded once, right after
            # the collective, ahead of the big MLP weight streams)
            kps = []
            for hp in range(H // 2):
                kp = kvp.tile([P, KC + 1, P], F8, tag="kp", bufs=8, name=f"kp{hp}")
                nc.sync.dma_start(
                    kp[:, 0:KC, :].rearrange("p (r tc) k -> p r (tc k)", r=RANKS),
                    kvo[:, hp, :, :].rearrange("r p t -> p r t"),
                )
                nc.gpsimd.memset(kp[:, KC : KC + 1, :], 0.0)
                kps.append(kp)

            for wc in range(2):
                nc.sync.dma_start(
                    wproj[:, 4 * wc : 4 * wc + 4, :], wproj_d[:, 4 * wc : 4 * wc + 4, :]
                )
            for hh in range(16):
                nc.sync.dma_start(
                    wfc1[:, :, hh * (HID // 16) : (hh + 1) * (HID // 16)],
                    wfc1_d[:, :, hh * (HID // 16) : (hh + 1) * (HID // 16)],
                )

            # V tiles resident per head pair: [keys_p, kc, 2*DH] (two heads'
            # 64-wide column blocks are contiguous in the collective buffer)
            vps = []
            for hp in range(H // 2):
                vp = vfp.tile([P, KC, P], F8, tag="vp", bufs=8, name=f"vp{hp}")
                vp4 = vp[:].rearrange("p (r tc) d -> p r tc d", r=RANKS)
                a0 = CH + (hp // 4) * (TOK // P)  # V half vh = hp // 4
                for r_ in range(RANKS):
                    nc.sync.dma_start(
                        vp4[:, r_, :, :],
                        kvo[r_, a0 : a0 + TOK // P, :,
                            (hp % 4) * P : (hp % 4 + 1) * P]
                        .rearrange("tc p d -> p tc d"),
                    )
                vps.append(vp)

            # ================= attention + MLP pipeline =================
            # Query quarters (128 tokens) pipeline attention's ACT-bound exp
            # stream against the PE-bound MLP; fc2 runs per half so its
            # weights stream only twice.
            x2 = res.tile([P, CH, TOK], F16, tag="x2")
            NQ = 4
            QQ = TOK // NQ  # 128
            app = (
                tc.tile_pool(name="ps_s", bufs=2, space="PSUM"),
                tc.tile_pool(name="ps_c", bufs=2, space="PSUM"),
                tc.tile_pool(name="ps_f1", bufs=2, space="PSUM"),
            )
            ps_s_p = app[0].__enter__()
            ps_c_p = app[1].__enter__()
            ps_f1 = app[2].__enter__()

            g8s = {}
            hmids = {}

            def attn_q(qq, hooks=None):
                qsl = slice(qq * QQ, (qq + 1) * QQ)
                xall = xc3.tile([P, CH, QQ], F32, tag="xa", bufs=2, name=f"xall{qq}")
                nc.sync.dma_start(xall[:], xT_chunks[:, :, qsl])
                ctxT = cxp.tile([P, CH, QQ], F8, tag="ctxT", name=f"ctxT{qq}")
                for h in range(H):
                    if hooks and h in hooks:
                        hooks[h]()
                    half = slice((h % 2) * DH, (h % 2) * DH + DH)
                    kp = kps[h // 2]
                    vp = vps[h // 2]
                    dsl = slice((h % 2) * DH, (h % 2) * DH + DH)
                    pt = cxp.tile([P, KC, QQ], F8, tag="pt", name=f"pt{qq}_{h}")
                    for g in range(2):
                        ps_s = ps_s_p.tile([P, 8, QQ], F32, tag="s", name=f"s{qq}_{h}_{g}")
                        for j in range(8):
                            kc = g * 8 + j
                            nc.tensor.matmul(
                                ps_s[:, j, :],
                                kp[half, kc : kc + 2, :],
                                qzT[half, h // 2 : CH + 1 : CH - h // 2, qsl],
                                start=True, stop=True, perf_mode=DR,
                            )
                        nc.scalar.activation(
                            out=pt[:, g * 8 : (g + 1) * 8, :],
                            in_=ps_s[:],
                            func=AF.Exp, scale=SCALE, bias=expb_row[:],
                        )
                    ps_c = ps_c_p.tile([DH, QQ], F32, tag="cx", name=f"c{qq}_{h}")
                    ps_d = ps_c_p.tile([1, QQ], F32, tag="cx", name=f"d{qq}_{h}")
                    for kc in range(0, KC, 2):
                        nc.tensor.matmul(
                            ps_c[:], vp[:, kc : kc + 2, dsl],
                            pt[:, kc : kc + 2, :],
                            start=(kc == 0), stop=(kc == KC - 2), perf_mode=DR,
                        )
                        nc.tensor.matmul(
                            ps_d[:], ones8d[:, :, 0:1],
                            pt[:, kc : kc + 2, :],
                            start=(kc == 0), stop=(kc == KC - 2), perf_mode=DR,
                        )
                    rr = rows.tile([1, QQ], F32, tag="rq", bufs=3, name=f"rr{qq}_{h}")
                    nc.vector.reciprocal(rr[:], ps_d[:])
                    rb = bcp.tile([DH, QQ], F32, tag="rb", bufs=2, name=f"rb{qq}_{h}")
                    nc.gpsimd.partition_broadcast(rb[:], rr[:])
                    nc.vector.tensor_tensor(
                        ctxT[half, h // 2, :], ps_c[:], rb[:], OP.mult
                    )

                # ---- proj + residual (fp8 DoubleRow) ----
                for m in range(CH):
                    ps = ps_f1.tile([P, QQ], F32, tag="f", name=f"pj{qq}_{m}")
                    for ch in range(0, CH, 2):
                        nc.tensor.matmul(
                            ps[:], wproj[:, ch : ch + 2, m * P : (m + 1) * P],
                            ctxT[:, ch : ch + 2, :],
                            start=(ch == 0), stop=(ch == CH - 2), perf_mode=DR,
                        )
                    nc.vector.scalar_tensor_tensor(
                        out=x2[:, m, qsl], in0=ps[:],
                        scalar=projb_sb[:, m : m + 1], in1=xall[:, m, :],
                        op0=OP.add, op1=OP.add,
                    )

            def mlp_q(qq):
                qsl = slice(qq * QQ, (qq + 1) * QQ)
                # ---- LN2 stats (fp16 ones-matmuls straight off x2) ----
                sq2 = b4k.tile([P, CH, QQ], F16, tag="b4k", name=f"sq2_{qq}")
                for ch in range(CH):
                    nc.vector.tensor_mul(sq2[:, ch, :], x2[:, ch, qsl], x2[:, ch, qsl])
                ps_r = ps_c_p.tile([33, QQ], F32, tag="cx", name=f"r2_{qq}")
                for ch in range(CH):
                    nc.tensor.matmul(
                        ps_r[0:1, :], ones16[:], x2[:, ch, qsl],
                        start=(ch == 0), stop=(ch == CH - 1),
                        skip_group_check=True,
                    )
                    nc.tensor.matmul(
                        ps_r[32:33, :], ones16[:], sq2[:, ch, :],
                        start=(ch == 0), stop=(ch == CH - 1),
                        skip_group_check=True,
                    )
                mu2 = rows.tile([1, QQ], F32, tag="rq", bufs=3, name=f"mu2_{qq}")
                nc.vector.tensor_scalar_mul(mu2[:], ps_r[0:1, :], 1.0 / D)
                var2 = rows.tile([1, QQ], F32, tag="rq", bufs=3, name=f"var2_{qq}")
                nc.vector.tensor_tensor(var2[:], mu2[:], mu2[:], OP.mult)
                ex22 = rows.tile([1, QQ], F32, tag="rq", bufs=3, name=f"ex22_{qq}")
                nc.vector.tensor_scalar_mul(ex22[:], ps_r[32:33, :], 1.0 / D)
                nc.vector.tensor_sub(var2[:], ex22[:], var2[:])
                nc.scalar.activation(out=ex22[:], in_=var2[:], func=AF.Ln, bias=eps_row[:])
                rstd2 = var2
                nc.scalar.activation(out=rstd2[:], in_=ex22[:], func=AF.Exp, scale=-0.5)
                nc.vector.tensor_tensor(mu2[:], mu2[:], rstd2[:], OP.mult)  # in place
                rstd2_b = bcp.tile([P, QQ], F32, tag="bc", name=f"rstd2b{qq}")
                nc.gpsimd.partition_broadcast(rstd2_b[:], rstd2[:])
                c2_b = bcp.tile([P, QQ], F32, tag="bc", name=f"c2b{qq}")
                nc.gpsimd.partition_broadcast(c2_b[:], mu2[:])

                xn2 = b4k.tile([P, CH, QQ], F16, tag="b4k", name=f"xn2_{qq}")
                for ch in range(CH):
                    t1 = stg.tile([P, QQ], F16, tag="t1", name=f"ln2t{qq}_{ch}")
                    nc.vector.tensor_tensor(t1[:], x2[:, ch, qsl], rstd2_b[:], OP.mult)
                    nc.vector.tensor_sub(xn2[:, ch, :], t1[:], c2_b[:])

                # ---- fc1 + batched gelu (fp16); g8 half-tile reuses the
                # (now dead) wqkv slot. fc1 psums drain to SBUF via DVE so
                # gelu runs in 16-m batches and the ACT engine only swaps its
                # exp<->gelu table a few times per kernel. ----
                if qq % 2 == 0:
                    g8s[qq // 2] = wbig.tile([P, HCH, 2 * QQ], F16, tag="wqkv", name=f"g8_{qq // 2}")
                for mg in range(2):
                    hmid = b4k.tile([P, 16, QQ], F16, tag="b4k", name=f"hm{qq}_{mg}")
                    hmids[(qq, mg)] = hmid
                    for mi in range(16):
                        m = mg * 16 + mi
                        ps = ps_f1.tile([P, QQ], F32, tag="f", name=f"f1_{qq}_{m}")
                        for ch in range(CH):
                            nc.tensor.matmul(
                                ps[:], wfc1[:, ch, m * P : (m + 1) * P], xn2[:, ch, :],
                                start=(ch == 0), stop=(ch == CH - 1),
                            )
                        nc.vector.tensor_scalar(
                            out=hmid[:, mi, :], in0=ps[:],
                            scalar1=fc1b_sb[:, m : m + 1], scalar2=None, op0=OP.add,
                        )

            def mlp_b(qq):
                g8 = g8s[qq // 2]
                gcol = slice((qq % 2) * QQ, (qq % 2) * QQ + QQ)
                for mg in range(2):
                    nc.scalar.activation(
                        out=g8[:, mg * 16 : (mg + 1) * 16, gcol],
                        in_=hmids[(qq, mg)][:],
                        func=AF.Gelu,
                    )

            def fc2_m2(hf, m2):
                    qq = 2 * hf + 1
                    g8 = g8s[hf]
                    hsl = slice(hf * 2 * QQ, hf * 2 * QQ + 2 * QQ)
                    if True:
                        w2a = w2p.tile([P, HCH // 2, P], F16, tag="w2", bufs=3, name=f"w2a_{qq}_{m2}")
                        nc.sync.dma_start(w2a[:], wfc2_d[m2, :, 0 : HCH // 2, :])
                        w2b = w2p.tile([P, HCH // 2, P], F16, tag="w2", bufs=3, name=f"w2b_{qq}_{m2}")
                        nc.sync.dma_start(w2b[:], wfc2_d[m2, :, HCH // 2 : HCH, :])
                        ps = ps_f1.tile([P, 2 * QQ], F32, tag="f", name=f"f2_{qq}_{m2}")
                        for hc in range(HCH):
                            w2t = w2a if hc < HCH // 2 else w2b
                            nc.tensor.matmul(
                                ps[:], w2t[:, hc % (HCH // 2), :], g8[:, hc, :],
                                start=(hc == 0), stop=(hc == HCH - 1),
                            )
                        o2 = stg.tile([P, 2 * QQ], F32, tag="o2", bufs=2, name=f"oo{qq}_{m2}")
                        nc.vector.scalar_tensor_tensor(
                            out=o2[:], in0=ps[:],
                            scalar=fc2b_sb[:, m2 : m2 + 1], in1=x2[:, m2, hsl],
                            op0=OP.add, op1=OP.add,
                        )
                        nc.sync.dma_start(outT[m2 * P : (m2 + 1) * P, hsl], o2[:])

            # software-pipelined emission: attention(qq+1) and the next
            # fc2 half carry a later scheduler priority than the exp stream
            # they must not starve.
            attn_q(0)
            attn_q(1)
            mlp_q(0)
            mlp_b(0)
            attn_q(2)
            mlp_q(1)
            mlp_b(1)
            for i in range(CH):
                fc2_m2(0, i)
            attn_q(3)
            mlp_q(2)
            mlp_b(2)
            mlp_q(3)
            mlp_b(3)
            for i in range(CH):
                fc2_m2(1, i)

            for pcm in reversed(app):
                pcm.__exit__(None, None, None)

    if do_compile:
        nc.compile()
    return nc


_CACHE = {}


def _get_program():
    if "nc" not in _CACHE:
        _CACHE["nc"] = build_program()
    return _CACHE["nc"]


def _prep_inputs(inputs):
    """Host-side sharding + layout prep. Returns per-core in_maps."""
    f32 = lambda k: np.asarray(inputs[k], np.float32)
    x = f32("x")
    ln1g, ln1b = f32("ln1_g"), f32("ln1_b")
    ln2g, ln2b = f32("ln2_g"), f32("ln2_b")
    qkv_w = f32("qkv_w") * ln1g[None, :]      # fold LN1 scale (exact)
    qb_full = f32("qkv_w") @ ln1b              # fold LN1 shift (exact)
    fc1_w = f32("fc1_w") * ln2g[None, :]
    fc1b = f32("fc1_b") + f32("fc1_w") @ ln2b

    def stripe(v, n):
        return np.ascontiguousarray(np.asarray(v, np.float32).reshape(n, P).T)

    wqkv = np.ascontiguousarray(
        qkv_w.T.reshape(CH, P, 3 * D).transpose(1, 0, 2).astype(NP_F8)
    )
    wproj = np.ascontiguousarray(
        f32("proj_w").T.reshape(CH, P, D).transpose(1, 0, 2).astype(NP_F8)
    )
    wfc1 = np.ascontiguousarray(
        fc1_w.T.reshape(CH, P, HID).transpose(1, 0, 2).astype(np.float16)
    )
    # fc2 lhsT tiles [m2][hid_p][hc][o128]
    wfc2 = np.ascontiguousarray(
        f32("fc2_w").T.reshape(HCH, P, CH, P).transpose(2, 1, 0, 3).astype(np.float16)
    )
    # V's LN1-shift bias is constant per channel, so it commutes through the
    # softmax average and folds exactly into the projection bias.
    projb = np.asarray(inputs["proj_b"], np.float32) + f32("proj_w") @ qb_full[2 * D :]
    shared = {
        "wqkv": wqkv, "wproj": wproj, "wfc1": wfc1, "wfc2": wfc2,
        "qb": stripe(qb_full[0:D], CH),
        "kb": stripe(qb_full[D : 2 * D], CH),
        "projb": stripe(projb, CH),
        "fc1b": stripe(fc1b, HCH),
        "fc2b": stripe(inputs["fc2_b"], CH),
    }
    in_maps = []
    for c in range(NCORES):
        b, blk = divmod(c, RANKS)
        xblk = x[b, blk * TOK : (blk + 1) * TOK, :]  # [TOK, D]
        m = dict(shared)
        m["xT"] = np.ascontiguousarray(xblk.T)  # [D, TOK]
        m["xT16"] = np.ascontiguousarray(
            xblk.T.reshape(CH, P, TOK).transpose(1, 0, 2).astype(np.float16)
        )
        in_maps.append(m)
    return in_maps


def _assemble(results):
    out = np.empty((B, N, D), dtype=np.float32)
    for c in range(NCORES):
        b, blk = divmod(c, RANKS)
        out[b, blk * TOK : (blk + 1) * TOK, :] = results[c]["outT"].T
    return out


def run_device(inputs, **kwargs):
    nc = _get_program()
    in_maps = _prep_inputs(inputs)
    res = run_bass_kernel_spmd(nc, in_maps, core_ids=list(range(NCORES)), **kwargs)
    return _assemble(res.results), res


def kernel(**inputs) -> np.ndarray:
    out, _ = run_device(inputs)
    return out


# revision 5
# speedup vs baseline: 1.0112x; 1.0112x over previous
"""Trainium2 Bass kernel for a pre-norm transformer block (MHA + MLP).

Sharding: sequence-parallel over 8 cores (batch b = core//4, token block
core%4, 512 tokens each). Weights replicated. One 4-rank AllGather per
batch group moves K+V (fp8, 1MB in / 4MB out).

Dataflow is feature-major end-to-end (channels on partitions, tokens on
the free axis), no on-chip transposes. Precision plan:
  - attention path in fp8e4m3 with DoubleRow matmuls (0.5 cycles/row):
    LN1 stats, QKV, scores (zero-padded pairs), P*V (true kc pairs), proj.
    Softmax averaging keeps the fp8 noise out of the residual stream.
  - MLP in fp16 (1 cycle/row at any tile size, halves weight DMA).
  - residual stream x2 kept in fp16/fp32; LN affines folded into the
    adjacent weights (exact); softmax exp uses scale=1/8, bias=-3 to stay
    inside e4m3 range; rstd = exp(-0.5*ln(var+eps)) so the Activation
    engine never leaves the exp/ln table during the softmax stream.
  - two query-half pipeline: attention(qh1) exps overlap MLP(qh0) matmuls.
"""
import sys

sys.path.insert(0, "/opt/trn_rl_repo")
import numpy as np
import ml_dtypes
import concourse.bass as bass
import concourse.mybir as mybir
import concourse.tile as tile
from concourse import bacc
from concourse.bass_utils import run_bass_kernel_spmd

# problem shapes (hardcoded per contract)
B, N, D = 2, 2048, 1024
H, DH = 16, 64
HID = 4096
NCORES = 8
TOK = (B * N) // NCORES  # 512 tokens per core
EPS = 1e-5
SCALE = DH**-0.5
P = 128
CH = D // P        # 8 channel chunks of the model dim
KC = N // P        # 16 key chunks of the full sequence
HCH = HID // P     # 32 hidden chunks
RANKS = 4          # per-batch replica group size
NQH = 2            # query halves (pipeline granularity)
QT = TOK // NQH    # 256 tokens per half
EXPB = -4.0        # softmax exp bias (softmax shift-invariant)

F32 = mybir.dt.float32
F16 = mybir.dt.float16
F8 = mybir.dt.float8e4
F8E5 = mybir.dt.float8e5
AF = mybir.ActivationFunctionType
OP = mybir.AluOpType
DR = mybir.MatmulPerfMode.DoubleRow

NP_F8 = ml_dtypes.float8_e4m3

REPLICA_GROUPS = [[0, 1, 2, 3], [4, 5, 6, 7]]


def build_program(do_compile=True):
    nc = bacc.Bacc("TRN2", target_bir_lowering=False, debug=False, num_devices=NCORES)

    # ---- kernel I/O ----
    xT = nc.dram_tensor("xT", [D, TOK], F32, kind="ExternalInput").ap()
    xT16_d = nc.dram_tensor("xT16", [P, CH, TOK], F16, kind="ExternalInput").ap()
    wqkv_d = nc.dram_tensor("wqkv", [P, CH, 3 * D], F8, kind="ExternalInput").ap()
    wproj_d = nc.dram_tensor("wproj", [P, CH, D], F8, kind="ExternalInput").ap()
    wfc1_d = nc.dram_tensor("wfc1", [P, CH, HID], F16, kind="ExternalInput").ap()
    wfc2_d = nc.dram_tensor("wfc2", [CH, P, HCH, P], F16, kind="ExternalInput").ap()
    qb_d = nc.dram_tensor("qb", [P, CH], F32, kind="ExternalInput").ap()
    kb_d = nc.dram_tensor("kb", [P, CH], F32, kind="ExternalInput").ap()
    projb_d = nc.dram_tensor("projb", [P, CH], F32, kind="ExternalInput").ap()
    fc1b_d = nc.dram_tensor("fc1b", [P, HCH], F32, kind="ExternalInput").ap()
    fc2b_d = nc.dram_tensor("fc2b", [P, CH], F32, kind="ExternalInput").ap()
    outT = nc.dram_tensor("outT", [D, TOK], F32, kind="ExternalOutput").ap()

    xT_chunks = xT.rearrange("(ch p) t -> p ch t", p=P)

    with tile.TileContext(nc) as tc:
        with (
            tc.tile_pool(name="consts", bufs=1) as consts,
            tc.tile_pool(name="wbig", bufs=1) as wbig,
            tc.tile_pool(name="w2p", bufs=2) as w2p,
            tc.tile_pool(name="b4k", bufs=3) as b4k,
            tc.tile_pool(name="res", bufs=1) as res,
            tc.tile_pool(name="kv", bufs=2) as kvp,
            tc.tile_pool(name="vfp", bufs=1) as vfp,
            tc.tile_pool(name="cxp", bufs=2) as cxp,
            tc.tile_pool(name="xc3", bufs=2) as xc3,
            tc.tile_pool(name="stg", bufs=2) as stg,
            tc.tile_pool(name="rows", bufs=2) as rows,
            tc.tile_pool(name="bc", bufs=2) as bcp,
            tc.tile_pool(name="dram", bufs=1, space="DRAM") as dram,
        ):
            # ---- constants ----
            ones8 = consts.tile([P, 1], F8, tag="ones8")
            nc.vector.memset(ones8[:], 1.0)
            eps_row = consts.tile([1, 1], F32, tag="eps")
            nc.vector.memset(eps_row[:], EPS)
            expb_row = consts.tile([P, 1], F32, tag="expb")
            nc.vector.memset(expb_row[:], EXPB)
            ones16 = consts.tile([P, 1], F16, tag="ones16")
            nc.vector.memset(ones16[:], 1.0)
            ones8d = consts.tile([P, 2, 16], F8, tag="ones8d")
            nc.vector.memset(ones8d[:], 1.0)
            qb_sb = consts.tile([P, CH], F32, tag="qb")
            kb_sb = consts.tile([P, CH], F32, tag="kb")
            projb_sb = consts.tile([P, CH], F32, tag="projb")
            fc1b_sb = consts.tile([P, HCH], F32, tag="fc1b")
            fc2b_sb = consts.tile([P, CH], F32, tag="fc2b")
            for t_, s_ in (
                (qb_sb, qb_d), (kb_sb, kb_d), (projb_sb, projb_d),
                (fc1b_sb, fc1b_d), (fc2b_sb, fc2b_d),
            ):
                nc.gpsimd.dma_start(t_[:], s_[:])

            # ---- resident weights (wproj/wfc1 DMAs issued later: they are
            # needed only ~100us in, and the DMA device is serialized) ----
            wqkv = wbig.tile([P, CH, 3 * D], F8, tag="wqkv")
            wproj = wbig.tile([P, CH, D], F8, tag="wproj")
            wfc1 = wbig.tile([P, CH, HID], F16, tag="wfc1")

            # ---- collective DRAM buffers (K rows 0:1024 c-major, V rows
            # 1024:2048 token-major in vout halves) ----
            kv_in = dram.tile([2 * D, TOK], F8, tag="kvin")
            kv_out = dram.tile([RANKS * 2 * D, TOK], F8, tag="kvout")
            kvi = kv_in[:].rearrange("(a p) t -> a p t", p=P)  # a = 16 blocks
            kvo = kv_out[:].rearrange("(r a p) t -> r a p t", r=RANKS, p=P)

            # ================= front: LN1 =================
            # x arrives once as a host-prepared fp16 copy in the exact SBUF
            # layout (one fat DMA instead of 16 chunk loads + copies)
            xf16 = res.tile([P, CH, TOK], F16, tag="xf16")
            nc.sync.dma_start(xf16[:], xT16_d[:])

            with tc.tile_pool(name="prow", bufs=2, space="PSUM") as prow:
                ps_mu = prow.tile([1, TOK], F32, tag="row", name="ps_mu1")
                ps_s2 = prow.tile([1, TOK], F32, tag="row", name="ps_s21")
                for ch in range(CH):
                    sq = stg.tile([P, TOK], F16, tag="t1", bufs=3, name=f"sq16_{ch}")
                    nc.vector.tensor_mul(sq[:], xf16[:, ch, :], xf16[:, ch, :])
                    nc.tensor.matmul(
                        ps_mu[:], ones16[:], xf16[:, ch, :],
                        start=(ch == 0), stop=(ch == CH - 1),
                    )
                    nc.tensor.matmul(
                        ps_s2[:], ones16[:], sq[:],
                        start=(ch == 0), stop=(ch == CH - 1),
                    )
                mu = rows.tile([1, TOK], F16, tag="r", bufs=5, name="mu1")
                nc.vector.tensor_scalar_mul(mu[:], ps_mu[:], 1.0 / D)
                var = rows.tile([1, TOK], F16, tag="r", bufs=5, name="var1")
                nc.vector.tensor_tensor(var[:], mu[:], mu[:], OP.mult)
                ex2 = rows.tile([1, TOK], F16, tag="r", bufs=5, name="ex21")
                nc.vector.tensor_scalar_mul(ex2[:], ps_s2[:], 1.0 / D)
                nc.vector.tensor_sub(var[:], ex2[:], var[:])
            # rstd = exp(-0.5*ln(var+eps)) — stays in the exp/ln ACT table
            nc.scalar.activation(out=ex2[:], in_=var[:], func=AF.Ln, bias=eps_row[:])
            rstd = var
            nc.scalar.activation(out=rstd[:], in_=ex2[:], func=AF.Exp, scale=-0.5)
            nc.vector.tensor_tensor(mu[:], mu[:], rstd[:], OP.mult)  # cpos in place
            rstd_b = bcp.tile([P, TOK], F16, tag="bc", name="rstdb1")
            nc.gpsimd.partition_broadcast(rstd_b[:], rstd[:])
            c_b = bcp.tile([P, TOK], F16, tag="bc", name="cb1")
            nc.gpsimd.partition_broadcast(c_b[:], mu[:])

            h1 = b4k.tile([P, CH, TOK], F8, tag="b4k", name="h1")
            for ch in range(CH):
                t1 = stg.tile([P, TOK], F16, tag="t1", bufs=3, name=f"h1t{ch}")
                eng = nc.vector if ch % 2 == 0 else nc.gpsimd
                eng.tensor_mul(t1[:], xf16[:, ch, :], rstd_b[:])
                eng.tensor_sub(h1[:, ch, :], t1[:], c_b[:])

            # ================= front: QKV (fp8 DoubleRow) =================
            # weight DMAs chunked to ~1-2us device holds so they cannot
            # head-of-line-block latency-critical loads
            for wc in range(4):
                nc.sync.dma_start(
                    wqkv[:, 2 * wc : 2 * wc + 2, :], wqkv_d[:, 2 * wc : 2 * wc + 2, :]
                )
            fpool_cm = tc.tile_pool(name="ps_f", bufs=8, space="PSUM")
            fps = fpool_cm.__enter__()

            def qkv_psum(col0, name):
                ps = fps.tile([P, TOK], F32, tag="acc", name=name)
                for ch in range(0, CH, 2):
                    nc.tensor.matmul(
                        ps[:], wqkv[:, ch : ch + 2, col0 : col0 + P],
                        h1[:, ch : ch + 2, :],
                        start=(ch == 0), stop=(ch == CH - 2), perf_mode=DR,
                    )
                return ps

            # V first (token-major), drains alternating DVE/ACT so neither
            # engine paces the collective's input
            for vh in range(2):
                for tt in range(TOK // P):
                    ps = fps.tile([P, TOK], F32, tag="acc", name=f"v{vh}_{tt}")
                    for ch in range(0, CH, 2):
                        nc.tensor.matmul(
                            ps[:],
                            h1[:, ch : ch + 2, tt * P : (tt + 1) * P],
                            wqkv[:, ch : ch + 2, 2 * D + vh * TOK : 2 * D + (vh + 1) * TOK],
                            start=(ch == 0), stop=(ch == CH - 2), perf_mode=DR,
                        )
                    vtmp = stg.tile([P, TOK], F8, tag="cp", bufs=6, name=f"vtmp{vh}_{tt}")
                    if tt % 2 == 0:
                        nc.scalar.activation(out=vtmp[:], in_=ps[:], func=AF.Identity)
                    else:
                        nc.vector.tensor_copy(out=vtmp[:], in_=ps[:])
                    nc.sync.dma_start(kvi[CH + vh * (TOK // P) + tt, :, :], vtmp[:])
            for m in range(CH):
                ps = qkv_psum(D + m * P, f"k{m}")
                ktmp = stg.tile([P, TOK], F8, tag="cp", bufs=6, name=f"ktmp{m}")
                if m % 2 == 0:
                    nc.scalar.activation(
                        out=ktmp[:], in_=ps[:], func=AF.Identity,
                        bias=kb_sb[:, m : m + 1],
                    )
                else:
                    nc.vector.tensor_scalar(
                        out=ktmp[:], in0=ps[:], scalar1=kb_sb[:, m : m + 1],
                        scalar2=None, op0=OP.add,
                    )
                nc.sync.dma_start(kvi[m, :, :], ktmp[:])
            nc.gpsimd.collective_compute(
                "AllGather", OP.bypass,
                ins=[kv_in[:].opt()], outs=[kv_out[:].opt()],
                replica_groups=REPLICA_GROUPS,
            )
            # Q into qzT (slots 0..7 = q per m-group, slot 8 = shared zeros
            # that kill the second DoubleRow member of the score matmuls)
            qzT = res.tile([P, CH + 1, TOK], F8, tag="qzT")
            nc.gpsimd.memset(qzT[:, CH, :], 0.0)
            for m in range(CH):
                ps = qkv_psum(m * P, f"q{m}")
                nc.scalar.activation(
                    out=qzT[:, m, :], in_=ps[:], func=AF.Identity,
                    bias=qb_sb[:, m : m + 1],
                )
            fpool_cm.__exit__(None, None, None)

            # K tiles resident for all query chunks (loaded once, right after
            # the collective, ahead of the big MLP weight streams)
            kps = []
            for hp in range(H // 2):
                kp = kvp.tile([P, KC + 1, P], F8, tag="kp", bufs=8, name=f"kp{hp}")
                nc.sync.dma_start(
                    kp[:, 0:KC, :].rearrange("p (r tc) k -> p r (tc k)", r=RANKS),
                    kvo[:, hp, :, :].rearrange("r p t -> p r t"),
                )
                nc.gpsimd.memset(kp[:, KC : KC + 1, :], 0.0)
                kps.append(kp)

            for wc in range(2):
                nc.sync.dma_start(
                    wproj[:, 4 * wc : 4 * wc + 4, :], wproj_d[:, 4 * wc : 4 * wc + 4, :]
                )
            for hh in range(16):
                nc.sync.dma_start(
                    wfc1[:, :, hh * (HID // 16) : (hh + 1) * (HID // 16)],
                    wfc1_d[:, :, hh * (HID // 16) : (hh + 1) * (HID // 16)],
                )

            # V tiles resident per head pair: [keys_p, kc, 2*DH] (two heads'
            # 64-wide column blocks are contiguous in the collective buffer)
            vps = []
            for hp in range(H // 2):
                vp = vfp.tile([P, KC, P], F8, tag="vp", bufs=8, name=f"vp{hp}")
                vp4 = vp[:].rearrange("p (r tc) d -> p r tc d", r=RANKS)
                a0 = CH + (hp // 4) * (TOK // P)  # V half vh = hp // 4
                for r_ in range(RANKS):
                    nc.sync.dma_start(
                        vp4[:, r_, :, :],
                        kvo[r_, a0 : a0 + TOK // P, :,
                            (hp % 4) * P : (hp % 4 + 1) * P]
                        .rearrange("tc p d -> p tc d"),
                    )
                vps.append(vp)

            # ================= attention + MLP pipeline =================
            # Query quarters (128 tokens) pipeline attention's ACT-bound exp
            # stream against the PE-bound MLP; fc2 runs per half so its
            # weights stream only twice.
            x2 = res.tile([P, CH, TOK], F16, tag="x2")
            NQ = 4
            QQ = TOK // NQ  # 128
            app = (
                tc.tile_pool(name="ps_s", bufs=2, space="PSUM"),
                tc.tile_pool(name="ps_c", bufs=2, space="PSUM"),
                tc.tile_pool(name="ps_f1", bufs=2, space="PSUM"),
            )
            ps_s_p = app[0].__enter__()
            ps_c_p = app[1].__enter__()
            ps_f1 = app[2].__enter__()

            g8s = {}
            hmids = {}

            def attn_q(qq, hooks=None):
                qsl = slice(qq * QQ, (qq + 1) * QQ)
                xall = xc3.tile([P, CH, QQ], F32, tag="xa", bufs=2, name=f"xall{qq}")
                nc.sync.dma_start(xall[:], xT_chunks[:, :, qsl])
                ctxT = cxp.tile([P, CH, QQ], F8, tag="ctxT", name=f"ctxT{qq}")
                for h in range(H):
                    if hooks and h in hooks:
                        hooks[h]()
                    half = slice((h % 2) * DH, (h % 2) * DH + DH)
                    kp = kps[h // 2]
                    vp = vps[h // 2]
                    dsl = slice((h % 2) * DH, (h % 2) * DH + DH)
                    pt = cxp.tile([P, KC, QQ], F8, tag="pt", bufs=3, name=f"pt{qq}_{h}")
                    for g in range(2):
                        ps_s = ps_s_p.tile([P, 8, QQ], F32, tag="s", name=f"s{qq}_{h}_{g}")
                        for j in range(8):
                            kc = g * 8 + j
                            nc.tensor.matmul(
                                ps_s[:, j, :],
                                kp[half, kc : kc + 2, :],
                                qzT[half, h // 2 : CH + 1 : CH - h // 2, qsl],
                                start=True, stop=True, perf_mode=DR,
                            )
                        nc.scalar.activation(
                            out=pt[:, g * 8 : (g + 1) * 8, :],
                            in_=ps_s[:],
                            func=AF.Exp, scale=SCALE, bias=expb_row[:],
                        )
                    ps_c = ps_c_p.tile([DH, QQ], F32, tag="cx", name=f"c{qq}_{h}")
                    ps_d = ps_c_p.tile([1, QQ], F32, tag="cx", name=f"d{qq}_{h}")
                    for kc in range(0, KC, 2):
                        nc.tensor.matmul(
                            ps_c[:], vp[:, kc : kc + 2, dsl],
                            pt[:, kc : kc + 2, :],
                            start=(kc == 0), stop=(kc == KC - 2), perf_mode=DR,
                        )
                        nc.tensor.matmul(
                            ps_d[:], ones8d[:, :, 0:1],
                            pt[:, kc : kc + 2, :],
                            start=(kc == 0), stop=(kc == KC - 2), perf_mode=DR,
                        )
                    rr = rows.tile([1, QQ], F32, tag="rq", bufs=3, name=f"rr{qq}_{h}")
                    nc.vector.reciprocal(rr[:], ps_d[:])
                    rb = bcp.tile([DH, QQ], F32, tag="rb", bufs=2, name=f"rb{qq}_{h}")
                    nc.gpsimd.partition_broadcast(rb[:], rr[:])
                    nc.vector.tensor_tensor(
                        ctxT[half, h // 2, :], ps_c[:], rb[:], OP.mult
                    )

                # ---- proj + residual (fp8 DoubleRow) ----
                for m in range(CH):
                    ps = ps_f1.tile([P, QQ], F32, tag="f", name=f"pj{qq}_{m}")
                    for ch in range(0, CH, 2):
                        nc.tensor.matmul(
                            ps[:], wproj[:, ch : ch + 2, m * P : (m + 1) * P],
                            ctxT[:, ch : ch + 2, :],
                            start=(ch == 0), stop=(ch == CH - 2), perf_mode=DR,
                        )
                    nc.vector.scalar_tensor_tensor(
                        out=x2[:, m, qsl], in0=ps[:],
                        scalar=projb_sb[:, m : m + 1], in1=xall[:, m, :],
                        op0=OP.add, op1=OP.add,
                    )

            def mlp_q(qq):
                qsl = slice(qq * QQ, (qq + 1) * QQ)
                # ---- LN2 stats (fp16 ones-matmuls straight off x2) ----
                sq2 = b4k.tile([P, CH, QQ], F16, tag="b4k", name=f"sq2_{qq}")
                for ch in range(CH):
                    nc.vector.tensor_mul(sq2[:, ch, :], x2[:, ch, qsl], x2[:, ch, qsl])
                ps_r = ps_c_p.tile([33, QQ], F32, tag="cx", name=f"r2_{qq}")
                for ch in range(CH):
                    nc.tensor.matmul(
                        ps_r[0:1, :], ones16[:], x2[:, ch, qsl],
                        start=(ch == 0), stop=(ch == CH - 1),
                        skip_group_check=True,
                    )
                    nc.tensor.matmul(
                        ps_r[32:33, :], ones16[:], sq2[:, ch, :],
                        start=(ch == 0), stop=(ch == CH - 1),
                        skip_group_check=True,
                    )
                mu2 = rows.tile([1, QQ], F32, tag="rq", bufs=3, name=f"mu2_{qq}")
                nc.vector.tensor_scalar_mul(mu2[:], ps_r[0:1, :], 1.0 / D)
                var2 = rows.tile([1, QQ], F32, tag="rq", bufs=3, name=f"var2_{qq}")
                nc.vector.tensor_tensor(var2[:], mu2[:], mu2[:], OP.mult)
                ex22 = rows.tile([1, QQ], F32, tag="rq", bufs=3, name=f"ex22_{qq}")
                nc.vector.tensor_scalar_mul(ex22[:], ps_r[32:33, :], 1.0 / D)
                nc.vector.tensor_sub(var2[:], ex22[:], var2[:])
                nc.scalar.activation(out=ex22[:], in_=var2[:], func=AF.Ln, bias=eps_row[:])
                rstd2 = var2
                nc.scalar.activation(out=rstd2[:], in_=ex22[:], func=AF.Exp, scale=-0.5)
                nc.vector.tensor_tensor(mu2[:], mu2[:], rstd2[:], OP.mult)  # in place
                rstd2_b = bcp.tile([P, QQ], F32, tag="bc", name=f"rstd2b{qq}")
                nc.gpsimd.partition_broadcast(rstd2_b[:], rstd2[:])
                c2_b = bcp.tile([P, QQ], F32, tag="bc", name=f"c2b{qq}")
                nc.gpsimd.partition_broadcast(c2_b[:], mu2[:])

                xn2 = b4k.tile([P, CH, QQ], F16, tag="b4k", name=f"xn2_{qq}")
                for ch in range(CH):
                    t1 = stg.tile([P, QQ], F16, tag="t1", bufs=3, name=f"ln2t{qq}_{ch}")
                    nc.vector.tensor_tensor(t1[:], x2[:, ch, qsl], rstd2_b[:], OP.mult)
                    nc.vector.tensor_sub(xn2[:, ch, :], t1[:], c2_b[:])

                # ---- fc1 + batched gelu (fp16); g8 half-tile reuses the
                # (now dead) wqkv slot. fc1 psums drain to SBUF via DVE so
                # gelu runs in 16-m batches and the ACT engine only swaps its
                # exp<->gelu table a few times per kernel. ----
                if qq % 2 == 0:
                    g8s[qq // 2] = wbig.tile([P, HCH, 2 * QQ], F16, tag="wqkv", name=f"g8_{qq // 2}")
                for mg in range(2):
                    hmid = b4k.tile([P, 16, QQ], F16, tag="b4k", name=f"hm{qq}_{mg}")
                    hmids[(qq, mg)] = hmid
                    for mi in range(16):
                        m = mg * 16 + mi
                        ps = ps_f1.tile([P, QQ], F32, tag="f", name=f"f1_{qq}_{m}")
                        for ch in range(CH):
                            nc.tensor.matmul(
                                ps[:], wfc1[:, ch, m * P : (m + 1) * P], xn2[:, ch, :],
                                start=(ch == 0), stop=(ch == CH - 1),
                            )
                        nc.vector.tensor_scalar(
                            out=hmid[:, mi, :], in0=ps[:],
                            scalar1=fc1b_sb[:, m : m + 1], scalar2=None, op0=OP.add,
                        )

            def mlp_b(qq):
                g8 = g8s[qq // 2]
                gcol = slice((qq % 2) * QQ, (qq % 2) * QQ + QQ)
                for mg in range(2):
                    nc.scalar.activation(
                        out=g8[:, mg * 16 : (mg + 1) * 16, gcol],
                        in_=hmids[(qq, mg)][:],
                        func=AF.Gelu,
                    )

            def fc2_m2(hf, m2):
                    qq = 2 * hf + 1
                    g8 = g8s[hf]
                    hsl = slice(hf * 2 * QQ, hf * 2 * QQ + 2 * QQ)
                    if True:
                        w2a = w2p.tile([P, HCH // 2, P], F16, tag="w2", bufs=3, name=f"w2a_{qq}_{m2}")
                        nc.sync.dma_start(w2a[:], wfc2_d[m2, :, 0 : HCH // 2, :])
                        w2b = w2p.tile([P, HCH // 2, P], F16, tag="w2", bufs=3, name=f"w2b_{qq}_{m2}")
                        nc.sync.dma_start(w2b[:], wfc2_d[m2, :, HCH // 2 : HCH, :])
                        ps = ps_f1.tile([P, 2 * QQ], F32, tag="f", name=f"f2_{qq}_{m2}")
                        for hc in range(HCH):
                            w2t = w2a if hc < HCH // 2 else w2b
                            nc.tensor.matmul(
                                ps[:], w2t[:, hc % (HCH // 2), :], g8[:, hc, :],
                                start=(hc == 0), stop=(hc == HCH - 1),
                            )
                        o2 = stg.tile([P, 2 * QQ], F32, tag="o2", bufs=2, name=f"oo{qq}_{m2}")
                        nc.vector.scalar_tensor_tensor(
                            out=o2[:], in0=ps[:],
                            scalar=fc2b_sb[:, m2 : m2 + 1], in1=x2[:, m2, hsl],
                            op0=OP.add, op1=OP.add,
                        )
                        nc.sync.dma_start(outT[m2 * P : (m2 + 1) * P, hsl], o2[:])

            # software-pipelined emission: attention(qq+1) and the next
            # fc2 half carry a later scheduler priority than the exp stream
            # they must not starve.
            attn_q(0)
            attn_q(1)
            mlp_q(0)
            mlp_b(0)
            attn_q(2)
            mlp_q(1)
            mlp_b(1)
            for i in range(CH):
                fc2_m2(0, i)
            attn_q(3)
            mlp_q(2)
            mlp_b(2)
            mlp_q(3)
            mlp_b(3)
            for i in range(CH):
                fc2_m2(1, i)

            for pcm in reversed(app):
                pcm.__exit__(None, None, None)

    if do_compile:
        nc.compile()
    return nc


_CACHE = {}


def _get_program():
    if "nc" not in _CACHE:
        _CACHE["nc"] = build_program()
    return _CACHE["nc"]


def _prep_inputs(inputs):
    """Host-side sharding + layout prep. Returns per-core in_maps."""
    f32 = lambda k: np.asarray(inputs[k], np.float32)
    x = f32("x")
    ln1g, ln1b = f32("ln1_g"), f32("ln1_b")
    ln2g, ln2b = f32("ln2_g"), f32("ln2_b")
    qkv_w = f32("qkv_w") * ln1g[None, :]      # fold LN1 scale (exact)
    qb_full = f32("qkv_w") @ ln1b              # fold LN1 shift (exact)
    fc1_w = f32("fc1_w") * ln2g[None, :]
    fc1b = f32("fc1_b") + f32("fc1_w") @ ln2b

    def stripe(v, n):
        return np.ascontiguousarray(np.asarray(v, np.float32).reshape(n, P).T)

    wqkv = np.ascontiguousarray(
        qkv_w.T.reshape(CH, P, 3 * D).transpose(1, 0, 2).astype(NP_F8)
    )
    wproj = np.ascontiguousarray(
        f32("proj_w").T.reshape(CH, P, D).transpose(1, 0, 2).astype(NP_F8)
    )
    wfc1 = np.ascontiguousarray(
        fc1_w.T.reshape(CH, P, HID).transpose(1, 0, 2).astype(np.float16)
    )
    # fc2 lhsT tiles [m2][hid_p][hc][o128]
    wfc2 = np.ascontiguousarray(
        f32("fc2_w").T.reshape(HCH, P, CH, P).transpose(2, 1, 0, 3).astype(np.float16)
    )
    # V's LN1-shift bias is constant per channel, so it commutes through the
    # softmax average and folds exactly into the projection bias.
    projb = np.asarray(inputs["proj_b"], np.float32) + f32("proj_w") @ qb_full[2 * D :]
    shared = {
        "wqkv": wqkv, "wproj": wproj, "wfc1": wfc1, "wfc2": wfc2,
        "qb": stripe(qb_full[0:D], CH),
        "kb": stripe(qb_full[D : 2 * D], CH),
        "projb": stripe(projb, CH),
        "fc1b": stripe(fc1b, HCH),
        "fc2b": stripe(inputs["fc2_b"], CH),
    }
    in_maps = []
    for c in range(NCORES):
        b, blk = divmod(c, RANKS)
        xblk = x[b, blk * TOK : (blk + 1) * TOK, :]  # [TOK, D]
        m = dict(shared)
        m["xT"] = np.ascontiguousarray(xblk.T)  # [D, TOK]
        m["xT16"] = np.ascontiguousarray(
            xblk.T.reshape(CH, P, TOK).transpose(1, 0, 2).astype(np.float16)
        )
        in_maps.append(m)
    return in_maps


def _assemble(results):
    out = np.empty((B, N, D), dtype=np.float32)
    for c in range(NCORES):
        b, blk = divmod(c, RANKS)
        out[b, blk * TOK : (blk + 1) * TOK, :] = results[c]["outT"].T
    return out


def run_device(inputs, **kwargs):
    nc = _get_program()
    in_maps = _prep_inputs(inputs)
    res = run_bass_kernel_spmd(nc, in_maps, core_ids=list(range(NCORES)), **kwargs)
    return _assemble(res.results), res


def kernel(**inputs) -> np.ndarray:
    out, _ = run_device(inputs)
    return out
